# revision 13
# baseline (speedup 1.0000x reference)
"""Trainium2 Bass kernel for nn_Actor (VAE encoder + residual MLP + heads).

Strategy: pure data parallel over 8 NeuronCores (512 samples each).
Activations are kept feature-major [D, 512] on-chip; all matmul operands are
fp16 (psum accumulates fp32), LayerNorm statistics are computed with
ones-vector reduction matmuls on the TensorEngine and applied with fused
scalar_tensor_tensor ops on the VectorEngine.
"""
import numpy as np

import concourse.bass as bass
import concourse.mybir as mybir
import concourse.tile as tile
from concourse.bass_utils import run_bass_kernel_spmd
from concourse.masks import make_identity
from concourse.vector_clock import ScopedClock

P = 128
NB = 512          # batch per core
NCORES = 8
F16 = mybir.dt.float16
F32 = mybir.dt.float32
AF = mybir.ActivationFunctionType
ALU = mybir.AluOpType
HALF_PI = float(np.float32(np.pi / 2))

# ---------------------------------------------------------------------------
# Walrus on this toolchain accepts at most ONE sync wait per instruction.
# Split multi-wait instructions by inserting single-wait NoOp carriers.
# ---------------------------------------------------------------------------
_MAX_WAITS = 1


def _split_json_waits(js, max_waits=_MAX_WAITS):
    n = 0
    for f in js.get("functions", []):
        for bb in f.get("blocks", []):
            new = []
            for inst in bb["instructions"]:
                si = inst.get("sync_info")
                waits = si.get("on_wait") if si else None
                if waits and len(waits) > max_waits:
                    eng = inst.get("engine")
                    head = waits[: len(waits) - max_waits]
                    si["on_wait"] = waits[len(waits) - max_waits:]
                    ctr = 0
                    while head:
                        chunk, head = head[:max_waits], head[max_waits:]
                        ctr += 1
                        n += 1
                        new.append({
                            "debug": inst.get("debug", 0),
                            "engine": eng, "ins": [], "outs": [],
                            "name": f"{inst['name']}-wsplit{ctr}",
                            "opcode": "NoOp",
                            "sync_info": {"on_wait": chunk, "on_update": []},
                        })
                new.append(inst)
            bb["instructions"] = new
    return n


_orig_to_json_bytes = None


def _patched_to_json_bytes(self):
    import orjson
    js = orjson.loads(_orig_to_json_bytes(self))
    _split_json_waits(js)
    return orjson.dumps(js)


def _patched_drain_and_barrier(self, tick_clock, wait_clock):
    nc = self.nc
    carrier = nc.sync.nop(nofuse=True)
    wait_clock.add_sem_waits(
        carrier.ins, ScopedClock({None: tick_clock.global_clock}))
    si = carrier.ins.sync_info
    waits = list(si.on_wait) if si is not None else []
    if len(waits) > _MAX_WAITS:
        carrier.ins.sync_info = mybir.SyncInfo(
            on_wait=waits[:_MAX_WAITS], on_update=list(si.on_update))
        rest = waits[_MAX_WAITS:]
        while rest:
            chunk, rest = rest[:_MAX_WAITS], rest[_MAX_WAITS:]
            extra = nc.sync.nop(nofuse=True)
            extra.ins.sync_info = mybir.SyncInfo(on_wait=chunk, on_update=[])
    nc.sync.drain()
    nc.all_engine_barrier()
    assert self.sems is not None
    popped = nc._tile_sem_poison_stack.pop()
    assert popped is self._sem_poison
    nc.clear_and_free_semaphores(list(self.sems.allocated().values()))
    nc.all_engine_barrier()


def _apply_patches():
    global _orig_to_json_bytes
    tile.TileContext._drain_and_barrier = _patched_drain_and_barrier
    if _orig_to_json_bytes is None:
        _orig_to_json_bytes = bass.Bass.to_json_bytes
        bass.Bass.to_json_bytes = _patched_to_json_bytes


# ---------------------------------------------------------------------------
# Layer table: name -> (K_padded, M).  Weights are stored [K_padded, M] fp16
# (i.e. W.T, zero-padded on K where needed).
# ---------------------------------------------------------------------------
LAYERS = {
    "fc1":  (640, 1024),
    "fc21": (1024, 2048),
    "fc22": (1024, 2048),
    "fc3":  (2048, 1024),
    "fc4":  (1024, 264),
    "r1f1": (2048, 8192),
    "r1f2": (8192, 2048),
    "r2f1": (2048, 4096),
    "r2f2": (4096, 2048),
    "fc":   (1024, 128),
}
# layers whose bias is applied by the ACT epilogue (packed [P, n_m] f32)
BIAS32_LAYERS = ["fc1", "fc3", "fc4", "r1f1", "r1f2", "r2f1", "r2f2", "fc"]
LNS = {"r1l1": 8192, "r1l2": 2048, "r2l1": 4096, "r2l2": 2048}

# output row offsets in outT [4368, NB]
ROW_ACTION = 0   # 3 rows
ROW_P20 = 3
ROW_P30 = 4
ROW_P40 = 5
ROW_P21 = 6
ROW_P22 = 7
ROW_RECON = 8    # 264 rows
ROW_MU = 272     # 2048 rows
ROW_LV = 2320    # 2048 rows


def _mm_linear(nc, wpool, psum_pool, w_dram, rhs3, K, M, epilogue,
               name, k_chunk=16, bias16=None, ones_row=None):
    """out-tile loop of a linear layer.

    rhs3: SBUF AP [P, K//P, NB] fp16 (feature-major activations).
    epilogue(t, mw, psum_ap) emits the ops consuming each out tile.
    bias16: optional [1, M] fp16 SBUF AP -> bias added via K=1 matmul
            (ones_row [1, NB] fp16 required).
    """
    n_k = K // P
    n_m = (M + P - 1) // P
    for t in range(n_m):
        mw = min(P, M - t * P)
        pt = psum_pool.tile([P, NB], F32, name=f"ps_{name}{t}", tag="mm")
        first = True
        for c0 in range(0, n_k, k_chunk):
            ck = min(k_chunk, n_k - c0)
            ws = wpool.tile([P, ck, P], F16, name=f"w_{name}{t}_{c0}",
                            tag=f"w_{ck}")
            nc.sync.dma_start(
                ws[:, :, :mw],
                w_dram[c0 * P:(c0 + ck) * P, t * P:t * P + mw]
                .rearrange("(kt p) m -> p kt m", p=P),
            )
            for kt in range(ck):
                last = (c0 + kt == n_k - 1) and bias16 is None
                nc.tensor.matmul(pt[:mw], lhsT=ws[:, kt, :mw],
                                 rhs=rhs3[:, c0 + kt, :],
                                 start=first, stop=last)
                first = False
        if bias16 is not None:
            bt = wpool.tile([1, P], F16, name=f"bt_{name}{t}", tag="bt")
            nc.sync.dma_start(bt[:, :mw], bias16[0:1, t * P:t * P + mw])
            nc.tensor.matmul(pt[:mw], lhsT=bt[:, :mw],
                             rhs=ones_row[:], start=False, stop=True)
        epilogue(t, mw, pt)


def _ln_stats_apply(nc, pool, pstat, pbc, psum_small, ystage, n_m, D,
                    g_ap, ng_ap, be_ap, ones16, s1, s2, name,
                    lrelu_out=True):
    """Finalize LN stats (s1/s2 psum [1, NB]) and apply LN in-place on ystage.

    ystage: [P, n_m, NB] fp16.  If lrelu_out, writes lrelu(ln(y)) fp16
    in-place; otherwise leaves (ln(y) - beta-pending) handled by caller
    (caller passes lrelu_out=False and finishes per-tile).
    Returns (S_b, T_b) fp16 [P, NB] broadcast tiles (for caller use when
    lrelu_out=False: caller must apply u1/u2 itself).
    """
    inv_d = 1.0 / D
    st = pool.tile([1, 2 * NB], F32, name=f"st_{name}", tag="lnstat", bufs=1)
    # st[:, :NB] = mean, st[:, NB:] = inv-std
    nc.vector.tensor_scalar_mul(st[:, :NB], s1[:], inv_d)
    v1 = pool.tile([1, NB], F32, name=f"v1_{name}", tag="lnv1", bufs=1)
    nc.vector.tensor_scalar_mul(v1[:], s2[:], inv_d)
    msq = pool.tile([1, NB], F32, name=f"msq_{name}", tag="lnmsq", bufs=1)
    nc.vector.tensor_mul(msq[:], st[:, :NB], st[:, :NB])
    nc.vector.tensor_sub(v1[:], v1[:], msq[:])          # var
    epsb = pool.tile([1, 1], F32, name=f"eps_{name}", tag="lneps", bufs=1)
    nc.vector.memset(epsb[:], 1e-5)
    nc.scalar.activation(v1[:], v1[:], AF.Sqrt, bias=epsb[:])
    nc.vector.reciprocal(st[:, NB:], v1[:])             # inv-std
    # ms = mean * inv-std  (overwrite mean slot)
    nc.vector.tensor_mul(st[:, :NB], st[:, :NB], st[:, NB:])
    # fp16 copies for broadcast matmul rhs
    st16 = pool.tile([1, 2 * NB], F16, name=f"st16_{name}", tag="lnstat16", bufs=1)
    nc.vector.tensor_copy(st16[:], st[:])
    ones1 = pool.tile([1, P], F16, name=f"o1_{name}", tag="ones1", bufs=1)
    nc.vector.memset(ones1[:], 1.0)
    pS = pbc.tile([P, NB], F32, name=f"pS_{name}", tag="bc")
    nc.tensor.matmul(pS[:], lhsT=ones1[:], rhs=st16[:, NB:], start=True,
                     stop=True)
    pT = pbc.tile([P, NB], F32, name=f"pT_{name}", tag="bc")
    nc.tensor.matmul(pT[:], lhsT=ones1[:], rhs=st16[:, :NB], start=True,
                     stop=True)
    S_b = pool.tile([P, NB], F16, name=f"Sb_{name}", tag="lnSb", bufs=1)
    nc.scalar.activation(S_b[:], pS[:], AF.Copy)
    T_b = pool.tile([P, NB], F16, name=f"Tb_{name}", tag="lnTb", bufs=1)
    nc.scalar.activation(T_b[:], pT[:], AF.Copy)

    if lrelu_out:
        for t in range(n_m):
            yt = ystage[:, t, :]
            nc.vector.scalar_tensor_tensor(
                yt, in0=yt, scalar=g_ap[:, t:t + 1], in1=S_b[:],
                op0=ALU.mult, op1=ALU.mult)
            nc.vector.scalar_tensor_tensor(
                yt, in0=T_b[:], scalar=ng_ap[:, t:t + 1], in1=yt,
                op0=ALU.mult, op1=ALU.add)
            nc.scalar.activation(yt, yt, AF.Lrelu, bias=be_ap[:, t:t + 1],
                                 alpha=0.01)
    return S_b, T_b


def build_graph():
    _apply_patches()
    nc = bass.Bass()

    state = nc.declare_dram_parameter("state", [NB, 264], F32, isOutput=False)
    epsT = nc.declare_dram_parameter("epsT", [2048, NB], F32, isOutput=False)
    W = {n: nc.declare_dram_parameter(f"w_{n}", list(LAYERS[n]), F16,
                                      isOutput=False) for n in LAYERS}
    wh1 = nc.declare_dram_parameter("w_h1", [P, 64], F16, isOutput=False)
    wh2 = nc.declare_dram_parameter("w_h2", [P, 5], F16, isOutput=False)
    # packed biases [P, n_m] f32 for ACT epilogues
    b32 = {}
    for n in BIAS32_LAYERS:
        nm = (LAYERS[n][1] + P - 1) // P
        b32[n] = nc.declare_dram_parameter(f"b_{n}", [P, nm], F32,
                                           isOutput=False)
    b2122 = nc.declare_dram_parameter("b2122", [1, 4096], F16, isOutput=False)
    bh1 = nc.declare_dram_parameter("bh1", [64, 1], F32, isOutput=False)
    bh2 = nc.declare_dram_parameter("bh2", [5, 1], F32, isOutput=False)
    lnp = {}
    for ln, D in LNS.items():
        nm = D // P
        for pfx in ("g", "ng", "be"):
            lnp[f"{pfx}_{ln}"] = nc.declare_dram_parameter(
                f"{pfx}_{ln}", [P, nm], F32, isOutput=False)
    outT = nc.declare_dram_parameter("outT", [4368, NB], F32, isOutput=True)

    from contextlib import ExitStack
    with tile.TileContext(nc) as tc, ExitStack() as es:
        cpool = es.enter_context(tc.tile_pool(name="const", bufs=1))
        wpool = es.enter_context(tc.tile_pool(name="wts", bufs=2))
        psum = es.enter_context(tc.tile_pool(name="psmm", bufs=4,
                                             space="PSUM"))
        pbc = es.enter_context(tc.tile_pool(name="psbc", bufs=2,
                                            space="PSUM"))
        pstat = es.enter_context(tc.tile_pool(name="psst", bufs=1,
                                              space="PSUM"))
        pmain = es.enter_context(tc.tile_pool(name="main", bufs=1))
        lv16 = pmain.tile([P, 16, NB], F16, name="lv16")
        x016 = pmain.tile([P, 16, NB], F16, name="x016")
        r2in = pmain.tile([P, 16, NB], F16, name="r2in")
        x2 = pmain.tile([P, 8, NB], F16, name="x2")

        # ---- constants ----
        ident = cpool.tile([P, P], F32, name="ident")
        make_identity(nc, ident[:])
        ones16 = cpool.tile([P, 1], F16, name="ones16")
        nc.vector.memset(ones16[:], 1.0)
        ones_row = cpool.tile([1, NB], F16, name="ones_row")
        nc.vector.memset(ones_row[:], 1.0)
        tb32 = {}
        for n in BIAS32_LAYERS:
            nm = (LAYERS[n][1] + P - 1) // P
            tb32[n] = cpool.tile([P, nm], F32, name=f"tb_{n}")
            nc.sync.dma_start(tb32[n][:], b32[n][:])
        tbh1 = cpool.tile([64, 1], F32, name="tbh1")
        nc.sync.dma_start(tbh1[:], bh1[:])
        tbh2 = cpool.tile([5, 1], F32, name="tbh2")
        nc.sync.dma_start(tbh2[:], bh2[:])
        tln = {}
        for k in lnp:
            shp = list(lnp[k].shape)
            tln[k] = cpool.tile(shp, F32, name=f"t{k}")
            nc.sync.dma_start(tln[k][:], lnp[k][:])
        twh1 = cpool.tile([P, 64], F16, name="twh1")
        nc.sync.dma_start(twh1[:], wh1[:])
        twh2 = cpool.tile([P, 5], F16, name="twh2")
        nc.sync.dma_start(twh2[:], wh2[:])

        # =================== encode ===================
        cmA = tc.tile_pool(name="pA", bufs=1)
        pA = cmA.__enter__()
        cmAt = tc.tile_pool(name="pAt", bufs=2)
        pAt = cmAt.__enter__()
        h0T = pA.tile([P, 5, NB], F16, name="h0T")
        nc.vector.memset(h0T[:, 4, :], 0.0)
        epse = cpool.tile([P, 1], F32, name="epse")
        nc.vector.memset(epse[:], 1e-12)
        for i in range(4):
            sv = pAt.tile([P, 264], F32, name=f"sv{i}", tag="sv")
            nc.sync.dma_start(sv[:], state[i * P:(i + 1) * P, :])
            h0 = pAt.tile([P, 528], F32, name=f"h0b{i}", tag="h0b")
            # stateVec
            nc.vector.tensor_copy(h0[:, 0:264], sv[:])
            # thickness even slots: x coords of pts 0..65
            t_ev_o = h0[:, 264:396].rearrange("p (i two) -> p i two", two=2)
            sv_x = sv[:, 0:132].rearrange("p (i two) -> p i two", two=2)
            nc.vector.tensor_copy(t_ev_o[:, :, 0:1], sv_x[:, :, 0:1])
            # thickness odd slots: |y_i - y_{i+66}|
            sv_y0 = sv[:, 0:132].rearrange("p (i two) -> p i two", two=2)
            sv_y1 = sv[:, 132:264].rearrange("p (i two) -> p i two", two=2)
            nc.vector.tensor_sub(t_ev_o[:, :, 1:2], sv_y0[:, :, 1:2],
                                 sv_y1[:, :, 1:2])
            nc.scalar.activation(t_ev_o[:, :, 1:2], t_ev_o[:, :, 1:2], AF.Abs)
            # vectors
            v = pAt.tile([P, 264], F32, name=f"v{i}", tag="v")
            nc.vector.tensor_sub(v[:, 0:262], sv[:, 2:264], sv[:, 0:262])
            nc.vector.tensor_sub(v[:, 262:264], sv[:, 0:2], sv[:, 262:264])
            # v2: gather of even vector indices
            v2 = pAt.tile([P, 264], F32, name=f"v2{i}", tag="v2")
            v_g = v[:, 4:264].rearrange("p (i four) -> p i four", four=4)
            for off in (0, 132):
                d = v2[:, off:off + 130].rearrange(
                    "p (i two) -> p i two", two=2)
                nc.vector.tensor_copy(d[:], v_g[:, :, 0:2])
                nc.vector.tensor_copy(v2[:, off + 130:off + 132], v[:, 0:2])
            # dot and norms
            pr = pAt.tile([P, 264], F32, name=f"pr{i}", tag="pr")
            nc.vector.tensor_mul(pr[:], v[:], v2[:])
            pr2 = pr.rearrange("p (i two) -> p i two", two=2)
            dot = pAt.tile([P, 132], F32, name=f"dot{i}", tag="dot")
            nc.vector.tensor_add(dot[:, :, None], pr2[:, :, 0:1], pr2[:, :, 1:2])
            nc.vector.tensor_mul(pr[:], v[:], v[:])
            n1 = pAt.tile([P, 132], F32, name=f"n1{i}", tag="n1")
            nc.vector.tensor_add(n1[:, :, None], pr2[:, :, 0:1], pr2[:, :, 1:2])
            n2 = pAt.tile([P, 132], F32, name=f"n2{i}", tag="n2")
            n1_g = n1[:, 2:132].rearrange("p (i two) -> p i two", two=2)
            for off in (0, 66):
                nc.vector.tensor_copy(n2[:, off:off + 65],
                                      n1_g[:, :, 0:1].rearrange(
                                          "p i one -> p (i one)"))
                nc.vector.tensor_copy(n2[:, off + 65:off + 66], n1[:, 0:1])
            den = pAt.tile([P, 132], F32, name=f"den{i}", tag="den")
            nc.vector.tensor_mul(den[:], n1[:], n2[:])
            nc.scalar.activation(den[:], den[:], AF.Sqrt, bias=epse[:])
            nc.vector.reciprocal(den[:], den[:])
            cosv = pAt.tile([P, 132], F32, name=f"cos{i}", tag="cos")
            nc.vector.tensor_mul(cosv[:], dot[:], den[:])
            nc.vector.tensor_scalar(out=cosv[:], in0=cosv[:], scalar1=1.0,
                                    scalar2=-1.0, op0=ALU.min, op1=ALU.max)
            # angles = pi/2 - atan(c / sqrt(1 - c^2))
            s1c = pAt.tile([P, 132], F32, name=f"s1c{i}", tag="s1c")
            nc.vector.tensor_mul(s1c[:], cosv[:], cosv[:])
            nc.vector.tensor_scalar(out=s1c[:], in0=s1c[:], scalar1=-1.0,
                                    scalar2=1.0, op0=ALU.mult, op1=ALU.add)
            nc.scalar.activation(s1c[:], s1c[:], AF.Sqrt, bias=epse[:])
            nc.vector.reciprocal(s1c[:], s1c[:])
            nc.vector.tensor_mul(s1c[:], cosv[:], s1c[:])
            nc.scalar.activation(s1c[:], s1c[:], AF.Arctan)
            nc.vector.tensor_scalar(out=h0[:, 396:528], in0=s1c[:],
                                    scalar1=-1.0, scalar2=HALF_PI,
                                    op0=ALU.mult, op1=ALU.add)
            # transpose h0 -> h0T
            for f in range(5):
                fw = 128 if f < 4 else 16
                ptp = pbc.tile([P, P], F32, name=f"ptp{i}_{f}", tag="bc")
                nc.tensor.transpose(ptp[:fw, :], h0[:, f * P:f * P + fw],
                                    ident[:])
                nc.scalar.activation(h0T[0:fw, f, i * P:(i + 1) * P],
                                     ptp[:fw, :], AF.Copy)

        # =================== fc1 -> h1 ===================
        h1 = pA.tile([P, 8, NB], F16, name="h1")

        def ep_fc1(t, mw, pt):
            nc.scalar.activation(h1[:, t, :], pt[:], AF.Gelu,
                                 bias=tb32["fc1"][:, t:t + 1])
        _mm_linear(nc, wpool, psum, W["fc1"], h0T, 640, 1024, ep_fc1, "fc1")

        # =================== fc21 (mu) / fc22 (logvar) + z ===================
        cmB = tc.tile_pool(name="pB", bufs=1)
        pB = cmB.__enter__()
        cmBt = tc.tile_pool(name="pBt", bufs=2)
        pBt = cmBt.__enter__()
        mu16 = pB.tile([P, 16, NB], F16, name="mu16")
        z16 = pB.tile([P, 16, NB], F16, name="z16")

        def ep_fc21(t, mw, pt):
            mo = pBt.tile([P, NB], F32, name=f"mo_{t}", tag="ocp")
            nc.scalar.activation(mo[:], pt[:], AF.Copy)
            nc.sync.dma_start(outT[ROW_MU + t * P:ROW_MU + (t + 1) * P, :],
                              mo[:])
            nc.vector.tensor_copy(mu16[:, t, :], mo[:])
        _mm_linear(nc, wpool, psum, W["fc21"], h1, 1024, 2048, ep_fc21,
                   "fc21", bias16=b2122[:, 0:2048], ones_row=ones_row)

        def ep_fc22(t, mw, pt):
            lo = pBt.tile([P, NB], F32, name=f"lo_{t}", tag="ocp")
            nc.scalar.activation(lo[:], pt[:], AF.Copy)
            nc.sync.dma_start(outT[ROW_LV + t * P:ROW_LV + (t + 1) * P, :],
                              lo[:])
            nc.vector.tensor_copy(lv16[:, t, :], lo[:])
            e05 = pBt.tile([P, NB], F32, name=f"e05_{t}", tag="e05")
            nc.scalar.activation(e05[:], pt[:], AF.Exp, scale=0.5)
            epst = pBt.tile([P, NB], F32, name=f"epst_{t}", tag="epst")
            nc.sync.dma_start(epst[:], epsT[t * P:(t + 1) * P, :])
            nc.vector.tensor_mul(e05[:], e05[:], epst[:])
            nc.vector.tensor_add(z16[:, t, :], e05[:], mu16[:, t, :])
        _mm_linear(nc, wpool, psum, W["fc22"], h1, 1024, 2048, ep_fc22,
                   "fc22", bias16=b2122[:, 2048:4096], ones_row=ones_row)

        # =================== decode: fc3 -> fc4 -> recon ===================
        g3 = pB.tile([P, 8, NB], F16, name="g3")

        def ep_fc3(t, mw, pt):
            nc.scalar.activation(g3[:, t, :], pt[:], AF.Gelu,
                                 bias=tb32["fc3"][:, t:t + 1])
        _mm_linear(nc, wpool, psum, W["fc3"], z16, 2048, 1024, ep_fc3, "fc3")

        def ep_fc4(t, mw, pt):
            rc = pBt.tile([P, NB], F32, name=f"rc{t}", tag="rc")
            nc.scalar.activation(rc[:mw, :], pt[:mw], AF.Sigmoid,
                                 bias=tb32["fc4"][:mw, t:t + 1])
            nc.sync.dma_start(
                outT[ROW_RECON + t * P:ROW_RECON + t * P + mw, :], rc[:mw, :])
        _mm_linear(nc, wpool, psum, W["fc4"], g3, 1024, 264, ep_fc4, "fc4")

        # release encode/decode pools (LIFO: pBt, pB, pAt, pA)
        cmBt.__exit__(None, None, None)
        cmB.__exit__(None, None, None)
        cmAt.__exit__(None, None, None)
        cmA.__exit__(None, None, None)

        # =================== resblock 1 ===================
        with tc.tile_pool(name="r1", bufs=1) as pr1, \
             tc.tile_pool(name="r1t", bufs=3) as pr1t:
            y1 = pr1.tile([P, 64, NB], F16, name="y1")
            s1 = pstat.tile([1, NB], F32, name="s1_r1l1", tag="s1")
            s2 = pstat.tile([1, NB], F32, name="s2_r1l1", tag="s2")

            def ep_r1f1(t, mw, pt):
                nc.scalar.activation(y1[:, t, :], pt[:], AF.Identity,
                                     bias=tb32["r1f1"][:, t:t + 1])
                y2 = pr1t.tile([P, NB], F16, name=f"y2a_{t}", tag="y2")
                nc.vector.tensor_mul(y2[:], y1[:, t, :], y1[:, t, :])
                nc.tensor.matmul(s1[:], lhsT=ones16[:], rhs=y1[:, t, :],
                                 start=(t == 0), stop=(t == 63))
                nc.tensor.matmul(s2[:], lhsT=ones16[:], rhs=y2[:],
                                 start=(t == 0), stop=(t == 63))
            _mm_linear(nc, wpool, psum, W["r1f1"], lv16, 2048, 8192,
                       ep_r1f1, "r1f1")
            _ln_stats_apply(nc, pr1t, pstat, pbc, None, y1, 64, 8192,
                            tln["g_r1l1"], tln["ng_r1l1"], tln["be_r1l1"],
                            ones16, s1, s2, "r1l1", lrelu_out=True)

            # r1f2 + ln2 + shortcut + lrelu + gelu -> x0, r2in
            yb = pr1.tile([P, 16, NB], F16, name="yb")
            s1b = pstat.tile([1, NB], F32, name="s1_r1l2", tag="s1")
            s2b = pstat.tile([1, NB], F32, name="s2_r1l2", tag="s2")

            def ep_r1f2(t, mw, pt):
                nc.scalar.activation(yb[:, t, :], pt[:], AF.Identity,
                                     bias=tb32["r1f2"][:, t:t + 1])
                y2 = pr1t.tile([P, NB], F16, name=f"y2b_{t}", tag="y2")
                nc.vector.tensor_mul(y2[:], yb[:, t, :], yb[:, t, :])
                nc.tensor.matmul(s1b[:], lhsT=ones16[:], rhs=yb[:, t, :],
                                 start=(t == 0), stop=(t == 15))
                nc.tensor.matmul(s2b[:], lhsT=ones16[:], rhs=y2[:],
                                 start=(t == 0), stop=(t == 15))
            _mm_linear(nc, wpool, psum, W["r1f2"], y1, 8192, 2048,
                       ep_r1f2, "r1f2")
            S_b, T_b = _ln_stats_apply(
                nc, pr1t, pstat, pbc, None, yb, 16, 2048,
                tln["g_r1l2"], tln["ng_r1l2"], tln["be_r1l2"],
                ones16, s1b, s2b, "r1l2", lrelu_out=False)

            for t in range(16):
                yt = yb[:, t, :]
                u = pr1t.tile([P, NB], F32, name=f"u_{t}", tag="u")
                nc.vector.scalar_tensor_tensor(
                    u[:], in0=yt, scalar=tln["g_r1l2"][:, t:t + 1],
                    in1=S_b[:], op0=ALU.mult, op1=ALU.mult)
                nc.vector.scalar_tensor_tensor(
                    u[:], in0=T_b[:], scalar=tln["ng_r1l2"][:, t:t + 1],
                    in1=u[:], op0=ALU.mult, op1=ALU.add)
                # shortcut: + logvar (fp16 copy), bias beta via ACT, lrelu
                nc.vector.tensor_add(u[:], u[:], lv16[:, t, :])
                nc.scalar.activation(u[:], u[:], AF.Lrelu,
                                     bias=tln["be_r1l2"][:, t:t + 1],
                                     alpha=0.01)
                x0f = pr1t.tile([P, NB], F32, name=f"x0f_{t}", tag="x0f")
                nc.scalar.activation(x0f[:], u[:], AF.Gelu)
                nc.vector.tensor_copy(x016[:, t, :], x0f[:])
                nc.vector.tensor_add(r2in[:, t, :], x0f[:], lv16[:, t, :])

        # =================== resblock 2 ===================
        with tc.tile_pool(name="r2", bufs=1) as pr2, \
             tc.tile_pool(name="r2t", bufs=3) as pr2t:
            ya = pr2.tile([P, 32, NB], F16, name="ya")
            s1c_ = pstat.tile([1, NB], F32, name="s1_r2l1", tag="s1")
            s2c_ = pstat.tile([1, NB], F32, name="s2_r2l1", tag="s2")

            def ep_r2f1(t, mw, pt):
                nc.scalar.activation(ya[:, t, :], pt[:], AF.Identity,
                                     bias=tb32["r2f1"][:, t:t + 1])
                y2 = pr2t.tile([P, NB], F16, name=f"y2c_{t}", tag="y2")
                nc.vector.tensor_mul(y2[:], ya[:, t, :], ya[:, t, :])
                nc.tensor.matmul(s1c_[:], lhsT=ones16[:], rhs=ya[:, t, :],
                                 start=(t == 0), stop=(t == 31))
                nc.tensor.matmul(s2c_[:], lhsT=ones16[:], rhs=y2[:],
                                 start=(t == 0), stop=(t == 31))
            _mm_linear(nc, wpool, psum, W["r2f1"], r2in, 2048, 4096,
                       ep_r2f1, "r2f1")
            _ln_stats_apply(nc, pr2t, pstat, pbc, None, ya, 32, 4096,
                            tln["g_r2l1"], tln["ng_r2l1"], tln["be_r2l1"],
                            ones16, s1c_, s2c_, "r2l1", lrelu_out=True)

            yd = pr2.tile([P, 16, NB], F16, name="yd")
            s1d = pstat.tile([1, NB], F32, name="s1_r2l2", tag="s1")
            s2d = pstat.tile([1, NB], F32, name="s2_r2l2", tag="s2")

            def ep_r2f2(t, mw, pt):
                nc.scalar.activation(yd[:, t, :], pt[:], AF.Identity,
                                     bias=tb32["r2f2"][:, t:t + 1])
                y2 = pr2t.tile([P, NB], F16, name=f"y2d_{t}", tag="y2")
                nc.vector.tensor_mul(y2[:], yd[:, t, :], yd[:, t, :])
                nc.tensor.matmul(s1d[:], lhsT=ones16[:], rhs=yd[:, t, :],
                                 start=(t == 0), stop=(t == 15))
                nc.tensor.matmul(s2d[:], lhsT=ones16[:], rhs=y2[:],
                                 start=(t == 0), stop=(t == 15))
            _mm_linear(nc, wpool, psum, W["r2f2"], ya, 4096, 2048,
                       ep_r2f2, "r2f2")
            S_b2, T_b2 = _ln_stats_apply(
                nc, pr2t, pstat, pbc, None, yd, 16, 2048,
                tln["g_r2l2"], tln["ng_r2l2"], tln["be_r2l2"],
                ones16, s1d, s2d, "r2l2", lrelu_out=False)

            # glu input = x1 + x0 where x1 = gelu(lrelu(ln2 + r2in))
            gsum = pr2.tile([P, 16, NB], F16, name="gsum")
            for t in range(16):
                yt = yd[:, t, :]
                u = pr2t.tile([P, NB], F32, name=f"u2_{t}", tag="u")
                nc.vector.scalar_tensor_tensor(
                    u[:], in0=yt, scalar=tln["g_r2l2"][:, t:t + 1],
                    in1=S_b2[:], op0=ALU.mult, op1=ALU.mult)
                nc.vector.scalar_tensor_tensor(
                    u[:], in0=T_b2[:], scalar=tln["ng_r2l2"][:, t:t + 1],
                    in1=u[:], op0=ALU.mult, op1=ALU.add)
                nc.vector.tensor_add(u[:], u[:], r2in[:, t, :])
                nc.scalar.activation(u[:], u[:], AF.Lrelu,
                                     bias=tln["be_r2l2"][:, t:t + 1],
                                     alpha=0.01)
                nc.scalar.activation(u[:], u[:], AF.Gelu)
                nc.vector.tensor_add(gsum[:, t, :], u[:], x016[:, t, :])

            # GLU: x2 = vals * sigmoid(gates)
            for t in range(8):
                sg = pr2t.tile([P, NB], F32, name=f"sg_{t}", tag="sg")
                nc.scalar.activation(sg[:], gsum[:, 8 + t, :], AF.Sigmoid)
                nc.vector.tensor_mul(x2[:, t, :], gsum[:, t, :], sg[:])

        # =================== fc -> x3, heads ===================
        ph = es.enter_context(tc.tile_pool(name="ph", bufs=1))
        x3 = ph.tile([P, 1, NB], F16, name="x3")

        def ep_fc(t, mw, pt):
            nc.scalar.activation(x3[:, 0, :], pt[:], AF.Gelu,
                                 bias=tb32["fc"][:, 0:1])
        _mm_linear(nc, wpool, psum, W["fc"], x2, 1024, 128, ep_fc, "fc")

        # heads level 1: [128 -> 64] (gelu rows 0..31, relu rows 32..63)
        hs16 = ph.tile([P, NB], F16, name="hs16")
        nc.vector.memset(hs16[:], 0.0)
        ph1 = pbc.tile([P, NB], F32, name="ph1", tag="bc")
        nc.tensor.matmul(ph1[:64, :], lhsT=twh1[:], rhs=x3[:, 0, :],
                         start=True, stop=True)
        nc.scalar.activation(hs16[0:32, :], ph1[0:32, :], AF.Gelu,
                             bias=tbh1[0:32, :])
        nc.scalar.activation(hs16[32:64, :], ph1[32:64, :], AF.Relu,
                             bias=tbh1[32:64, :])

        # softmax over rows 0..2
        e3 = ph.tile([3, NB], F32, name="e3")
        nc.scalar.activation(e3[:], hs16[0:3, :], AF.Exp)
        ones3 = cpool.tile([3, 1], F32, name="ones3")
        nc.vector.memset(ones3[:], 1.0)
        pS3 = pbc.tile([P, NB], F32, name="pS3", tag="bc")
        nc.tensor.matmul(pS3[0:1, :], lhsT=ones3[:], rhs=e3[:],
                         start=True, stop=True)
        rS = ph.tile([1, NB], F32, name="rS")
        nc.vector.reciprocal(rS[:], pS3[0:1, :])
        ones13 = cpool.tile([1, 3], F32, name="ones13")
        nc.vector.memset(ones13[:], 1.0)
        pB3 = pbc.tile([P, NB], F32, name="pB3", tag="bc")
        nc.tensor.matmul(pB3[0:3, :], lhsT=ones13[:], rhs=rS[:],
                         start=True, stop=True)
        act3 = ph.tile([3, NB], F32, name="act3")
        nc.vector.tensor_mul(act3[:], e3[:], pB3[0:3, :])
        nc.sync.dma_start(outT[ROW_ACTION:ROW_ACTION + 3, :], act3[:])

        # heads level 2: [128(padded 48) -> 5]
        ph2 = pbc.tile([P, NB], F32, name="ph2", tag="bc")
        nc.tensor.matmul(ph2[0:5, :], lhsT=twh2[:], rhs=hs16[:],
                         start=True, stop=True)
        # row 0: p2_0 = psig(t) = s*(1.1 - 0.1 s);  rows 1..4: C*ptanh
        # (compute both paths on all 5 rows: partition starts must be 0)
        sgm = ph.tile([5, NB], F32, name="sgm")
        nc.scalar.activation(sgm[:], ph2[0:5, :], AF.Sigmoid,
                             bias=tbh2[0:5, :])
        q0 = ph.tile([5, NB], F32, name="q0")
        nc.vector.tensor_scalar(out=q0[:], in0=sgm[:], scalar1=-0.1,
                                scalar2=1.1, op0=ALU.mult, op1=ALU.add)
        nc.vector.tensor_mul(q0[:], q0[:], sgm[:])
        nc.sync.dma_start(outT[ROW_P20:ROW_P20 + 1, :], q0[0:1, :])
        th = ph.tile([5, NB], F32, name="th")
        nc.scalar.activation(th[:], ph2[0:5, :], AF.Tanh, bias=tbh2[0:5, :])
        q1 = ph.tile([5, NB], F32, name="q1")
        nc.vector.tensor_scalar(out=q1[:], in0=th[:], scalar1=-0.0002,
                                scalar2=0.002, op0=ALU.mult, op1=ALU.add)
        nc.vector.tensor_mul(q1[:], q1[:], th[:])
        nc.sync.dma_start(outT[ROW_P30:ROW_P30 + 4, :], q1[1:5, :])

    return nc


# ---------------------------------------------------------------------------
# Host side
# ---------------------------------------------------------------------------
def _pack_bias(b, n_m):
    v = np.zeros((P, n_m), np.float32)
    flat = np.asarray(b, np.float32)
    for t in range(n_m):
        w = min(P, flat.shape[0] - t * P)
        v[:w, t] = flat[t * P:t * P + w]
    return v


def _prep_params(p):
    f16 = np.float16
    d = {}

    def wt(src, kpad=None):
        w = np.asarray(p[src + "_w"], np.float32).T  # [in, out]
        if kpad is not None and w.shape[0] < kpad:
            w = np.vstack([w, np.zeros((kpad - w.shape[0], w.shape[1]),
                                       np.float32)])
        return np.ascontiguousarray(w).astype(f16)

    d["w_fc1"] = wt("fc1", 640)
    d["w_fc21"] = wt("fc21")
    d["w_fc22"] = wt("fc22")
    d["w_fc3"] = wt("fc3")
    d["w_fc4"] = wt("fc4")
    d["w_r1f1"] = wt("r1_fc1")
    d["w_r1f2"] = wt("r1_fc2")
    d["w_r2f1"] = wt("r2_fc1")
    d["w_r2f2"] = wt("r2_fc2")
    d["w_fc"] = wt("fc")
    for n, src in [("fc1", "fc1"), ("fc3", "fc3"), ("fc4", "fc4"),
                   ("r1f1", "r1_fc1"), ("r1f2", "r1_fc2"),
                   ("r2f1", "r2_fc1"), ("r2f2", "r2_fc2"), ("fc", "fc")]:
        n_m = (LAYERS[n][1] + P - 1) // P
        d[f"b_{n}"] = _pack_bias(p[src + "_b"], n_m)
    d["b2122"] = np.concatenate(
        [np.asarray(p["fc21_b"], np.float32),
         np.asarray(p["fc22_b"], np.float32)])[None, :].astype(f16)
    # LN params
    for ln, src in [("r1l1", "r1_ln1"), ("r1l2", "r1_ln2"),
                    ("r2l1", "r2_ln1"), ("r2l2", "r2_ln2")]:
        D = LNS[ln]
        g = np.asarray(p[src + "_g"], np.float32)
        be = np.asarray(p[src + "_b"], np.float32)
        d[f"g_{ln}"] = _pack_bias(g, D // P)
        d[f"ng_{ln}"] = -d[f"g_{ln}"]
        d[f"be_{ln}"] = _pack_bias(be, D // P)
    # heads: fused level-1 weight [128, 51] = [act(3) | p1_0 | p1_1 | p1_2]
    wh1 = np.zeros((P, 64), np.float32)
    wh1[:, 0:3] = np.asarray(p["act_w"], np.float32).T
    wh1[:, 3:19] = np.asarray(p["p1_0_w"], np.float32).T
    wh1[:, 32:48] = np.asarray(p["p1_1_w"], np.float32).T
    wh1[:, 48:64] = np.asarray(p["p1_2_w"], np.float32).T
    d["w_h1"] = wh1.astype(f16)
    bh1 = np.zeros((64,), np.float32)
    bh1[0:3] = np.asarray(p["act_b"], np.float32)
    bh1[3:19] = np.asarray(p["p1_0_b"], np.float32)
    bh1[32:48] = np.asarray(p["p1_1_b"], np.float32)
    bh1[48:64] = np.asarray(p["p1_2_b"], np.float32)
    d["bh1"] = bh1[:, None].astype(np.float32)
    # level-2 block diagonal [128, 5]; rows 3..18 p1_0, 19..34 p1_1, 35..50 p1_2
    wh2 = np.zeros((P, 5), np.float32)
    wh2[3:19, 0] = np.asarray(p["p2_0_w"], np.float32)[0]
    wh2[3:19, 1] = np.asarray(p["p3_0_w"], np.float32)[0]
    wh2[3:19, 2] = np.asarray(p["p4_0_w"], np.float32)[0]
    wh2[32:48, 3] = np.asarray(p["p2_1_w"], np.float32)[0]
    wh2[48:64, 4] = np.asarray(p["p2_2_w"], np.float32)[0]
    d["w_h2"] = wh2.astype(f16)
    bh2 = np.array([p["p2_0_b"][0], p["p3_0_b"][0], p["p4_0_b"][0],
                    p["p2_1_b"][0], p["p2_2_b"][0]], np.float32)
    d["bh2"] = bh2[:, None]
    return d


_CACHED = {}


def kernel(state, eps, params):
    state = np.asarray(state, np.float32)
    eps = np.asarray(eps, np.float32)
    B = state.shape[0]
    per = B // NCORES
    assert per == NB, (B, NB)

    shared = _prep_params(params)
    in_maps = []
    for i in range(NCORES):
        m = dict(shared)
        m["state"] = np.ascontiguousarray(
            state[i * per:(i + 1) * per].reshape(per, 264))
        m["epsT"] = np.ascontiguousarray(eps[i * per:(i + 1) * per].T)
        in_maps.append(m)

    if "nc" not in _CACHED:
        _CACHED["nc"] = build_graph()
    nc = _CACHED["nc"]
    res = run_bass_kernel_spmd(nc, in_maps, core_ids=list(range(NCORES)))
    out = np.empty((B, 4368), np.float32)
    for i in range(NCORES):
        out[i * per:(i + 1) * per] = res.results[i]["outT"].T
    return out


# revision 15
# speedup vs baseline: 1.1724x; 1.1724x over previous
"""Trainium2 Bass kernel for nn_Actor (VAE encoder + residual MLP + heads).

Strategy: pure data parallel over 8 NeuronCores (512 samples each).
Activations are kept feature-major [D, 512] on-chip; all matmul operands are
fp16 (psum accumulates fp32), LayerNorm statistics are computed with
ones-vector reduction matmuls on the TensorEngine and applied with fused
scalar_tensor_tensor ops on the VectorEngine.
"""
import numpy as np

import concourse.bass as bass
import concourse.mybir as mybir
import concourse.tile as tile
from concourse.bass_utils import run_bass_kernel_spmd
from concourse.masks import make_identity
from concourse.vector_clock import ScopedClock

P = 128
NB = 512          # batch per core
NCORES = 8
F16 = mybir.dt.float16
F32 = mybir.dt.float32
AF = mybir.ActivationFunctionType
ALU = mybir.AluOpType
HALF_PI = float(np.float32(np.pi / 2))

# ---------------------------------------------------------------------------
# Walrus on this toolchain accepts at most ONE sync wait per instruction.
# Split multi-wait instructions by inserting single-wait NoOp carriers.
# ---------------------------------------------------------------------------
_MAX_WAITS = 1


def _split_json_waits(js, max_waits=_MAX_WAITS):
    n = 0
    for f in js.get("functions", []):
        for bb in f.get("blocks", []):
            new = []
            for inst in bb["instructions"]:
                si = inst.get("sync_info")
                waits = si.get("on_wait") if si else None
                if waits and len(waits) > max_waits:
                    eng = inst.get("engine")
                    head = waits[: len(waits) - max_waits]
                    si["on_wait"] = waits[len(waits) - max_waits:]
                    ctr = 0
                    while head:
                        chunk, head = head[:max_waits], head[max_waits:]
                        ctr += 1
                        n += 1
                        new.append({
                            "debug": inst.get("debug", 0),
                            "engine": eng, "ins": [], "outs": [],
                            "name": f"{inst['name']}-wsplit{ctr}",
                            "opcode": "NoOp",
                            "sync_info": {"on_wait": chunk, "on_update": []},
                        })
                new.append(inst)
            bb["instructions"] = new
    return n


_orig_to_json_bytes = None


def _patched_to_json_bytes(self):
    import orjson
    js = orjson.loads(_orig_to_json_bytes(self))
    _split_json_waits(js)
    return orjson.dumps(js)


def _patched_drain_and_barrier(self, tick_clock, wait_clock):
    nc = self.nc
    carrier = nc.sync.nop(nofuse=True)
    wait_clock.add_sem_waits(
        carrier.ins, ScopedClock({None: tick_clock.global_clock}))
    si = carrier.ins.sync_info
    waits = list(si.on_wait) if si is not None else []
    if len(waits) > _MAX_WAITS:
        carrier.ins.sync_info = mybir.SyncInfo(
            on_wait=waits[:_MAX_WAITS], on_update=list(si.on_update))
        rest = waits[_MAX_WAITS:]
        while rest:
            chunk, rest = rest[:_MAX_WAITS], rest[_MAX_WAITS:]
            extra = nc.sync.nop(nofuse=True)
            extra.ins.sync_info = mybir.SyncInfo(on_wait=chunk, on_update=[])
    nc.sync.drain()
    nc.all_engine_barrier()
    assert self.sems is not None
    popped = nc._tile_sem_poison_stack.pop()
    assert popped is self._sem_poison
    nc.clear_and_free_semaphores(list(self.sems.allocated().values()))
    nc.all_engine_barrier()


def _apply_patches():
    global _orig_to_json_bytes
    tile.TileContext._drain_and_barrier = _patched_drain_and_barrier
    if _orig_to_json_bytes is None:
        _orig_to_json_bytes = bass.Bass.to_json_bytes
        bass.Bass.to_json_bytes = _patched_to_json_bytes


# ---------------------------------------------------------------------------
# Layer table: name -> (K_padded, M).  Weights are stored [K_padded, M] fp16
# (i.e. W.T, zero-padded on K where needed).
# ---------------------------------------------------------------------------
LAYERS = {
    "fc1":  (640, 1024),
    "fc21": (1024, 2048),
    "fc22": (1024, 2048),
    "fc3":  (2048, 1024),
    "fc4":  (1024, 264),
    "r1f1": (2048, 8192),
    "r1f2": (8192, 2048),
    "r2f1": (2048, 4096),
    "r2f2": (4096, 2048),
    "fc":   (1024, 128),
}
# layers whose bias is applied by the ACT epilogue (packed [P, n_m] f32)
BIAS32_LAYERS = ["fc1", "fc3", "fc4", "r1f1", "r1f2", "r2f1", "r2f2", "fc"]
LNS = {"r1l1": 8192, "r1l2": 2048, "r2l1": 4096, "r2l2": 2048}

# output row offsets in outT [4368, NB]
ROW_ACTION = 0   # 3 rows
ROW_P20 = 3
ROW_P30 = 4
ROW_P40 = 5
ROW_P21 = 6
ROW_P22 = 7
ROW_RECON = 8    # 264 rows
ROW_MU = 272     # 2048 rows
ROW_LV = 2320    # 2048 rows


def _mm_linear(nc, wpool, psum_pool, w_dram, rhs3, K, M, epilogue,
               name, k_chunk=16, bias16=None, ones_row=None):
    """out-tile loop of a linear layer.

    rhs3: SBUF AP [P, K//P, NB] fp16 (feature-major activations).
    epilogue(t, mw, psum_ap) emits the ops consuming each out tile.
    bias16: optional [1, M] fp16 SBUF AP -> bias added via K=1 matmul
            (ones_row [1, NB] fp16 required).
    """
    n_k = K // P
    n_m = (M + P - 1) // P
    for t in range(n_m):
        mw = min(P, M - t * P)
        pt = psum_pool.tile([P, NB], F32, name=f"ps_{name}{t}", tag="mm")
        first = True
        for c0 in range(0, n_k, k_chunk):
            ck = min(k_chunk, n_k - c0)
            ws = wpool.tile([P, ck, P], F16, name=f"w_{name}{t}_{c0}",
                            tag=f"w_{ck}")
            nc.sync.dma_start(
                ws[:],
                w_dram[t, :, c0 * P:(c0 + ck) * P]
                .rearrange("p (kt m) -> p kt m", m=P),
            )
            for kt in range(ck):
                last = (c0 + kt == n_k - 1) and bias16 is None
                nc.tensor.matmul(pt[:mw], lhsT=ws[:, kt, :mw],
                                 rhs=rhs3[:, c0 + kt, :],
                                 start=first, stop=last)
                first = False
        if bias16 is not None:
            bt = wpool.tile([1, P], F16, name=f"bt_{name}{t}", tag="bt")
            nc.sync.dma_start(bt[:, :mw], bias16[0:1, t * P:t * P + mw])
            nc.tensor.matmul(pt[:mw], lhsT=bt[:, :mw],
                             rhs=ones_row[:], start=False, stop=True)
        epilogue(t, mw, pt)


def _ln_stats_apply(nc, pool, pstat, pbc, psum_small, ystage, n_m, D,
                    g_ap, ng_ap, be_ap, ones16, s1, s2, name,
                    lrelu_out=True):
    """Finalize LN stats (s1/s2 psum [1, NB]) and apply LN in-place on ystage.

    ystage: [P, n_m, NB] fp16.  If lrelu_out, writes lrelu(ln(y)) fp16
    in-place; otherwise leaves (ln(y) - beta-pending) handled by caller
    (caller passes lrelu_out=False and finishes per-tile).
    Returns (S_b, T_b) fp16 [P, NB] broadcast tiles (for caller use when
    lrelu_out=False: caller must apply u1/u2 itself).
    """
    inv_d = 1.0 / D
    st = pool.tile([1, 2 * NB], F32, name=f"st_{name}", tag="lnstat", bufs=1)
    # st[:, :NB] = mean, st[:, NB:] = inv-std
    nc.vector.tensor_scalar_mul(st[:, :NB], s1[:], inv_d)
    v1 = pool.tile([1, NB], F32, name=f"v1_{name}", tag="lnv1", bufs=1)
    nc.vector.tensor_scalar_mul(v1[:], s2[:], inv_d)
    msq = pool.tile([1, NB], F32, name=f"msq_{name}", tag="lnmsq", bufs=1)
    nc.vector.tensor_mul(msq[:], st[:, :NB], st[:, :NB])
    nc.vector.tensor_sub(v1[:], v1[:], msq[:])          # var
    epsb = pool.tile([1, 1], F32, name=f"eps_{name}", tag="lneps", bufs=1)
    nc.vector.memset(epsb[:], 1e-5)
    nc.scalar.activation(v1[:], v1[:], AF.Sqrt, bias=epsb[:])
    nc.vector.reciprocal(st[:, NB:], v1[:])             # inv-std
    # ms = mean * inv-std  (overwrite mean slot)
    nc.vector.tensor_mul(st[:, :NB], st[:, :NB], st[:, NB:])
    # fp16 copies for broadcast matmul rhs
    st16 = pool.tile([1, 2 * NB], F16, name=f"st16_{name}", tag="lnstat16", bufs=1)
    nc.vector.tensor_copy(st16[:], st[:])
    ones1 = pool.tile([1, P], F16, name=f"o1_{name}", tag="ones1", bufs=1)
    nc.vector.memset(ones1[:], 1.0)
    pS = pbc.tile([P, NB], F32, name=f"pS_{name}", tag="bc")
    nc.tensor.matmul(pS[:], lhsT=ones1[:], rhs=st16[:, NB:], start=True,
                     stop=True)
    pT = pbc.tile([P, NB], F32, name=f"pT_{name}", tag="bc")
    nc.tensor.matmul(pT[:], lhsT=ones1[:], rhs=st16[:, :NB], start=True,
                     stop=True)
    S_b = pool.tile([P, NB], F16, name=f"Sb_{name}", tag="lnSb", bufs=1)
    nc.scalar.activation(S_b[:], pS[:], AF.Copy)
    T_b = pool.tile([P, NB], F16, name=f"Tb_{name}", tag="lnTb", bufs=1)
    nc.scalar.activation(T_b[:], pT[:], AF.Copy)

    if lrelu_out:
        for t in range(n_m):
            yt = ystage[:, t, :]
            nc.vector.scalar_tensor_tensor(
                yt, in0=yt, scalar=g_ap[:, t:t + 1], in1=S_b[:],
                op0=ALU.mult, op1=ALU.mult)
            nc.vector.scalar_tensor_tensor(
                yt, in0=T_b[:], scalar=ng_ap[:, t:t + 1], in1=yt,
                op0=ALU.mult, op1=ALU.add)
            nc.scalar.activation(yt, yt, AF.Lrelu, bias=be_ap[:, t:t + 1],
                                 alpha=0.01)
    return S_b, T_b


def build_graph():
    _apply_patches()
    nc = bass.Bass()

    state = nc.declare_dram_parameter("state", [NB, 264], F32, isOutput=False)
    epsT = nc.declare_dram_parameter("epsT", [2048, NB], F32, isOutput=False)
    W = {}
    for n, (K, M) in LAYERS.items():
        nm = (M + P - 1) // P
        nk = K // P
        W[n] = nc.declare_dram_parameter(f"w_{n}", [nm, P, nk * P], F16,
                                         isOutput=False)
    wh1 = nc.declare_dram_parameter("w_h1", [P, 64], F16, isOutput=False)
    wh2 = nc.declare_dram_parameter("w_h2", [P, 5], F16, isOutput=False)
    # packed biases [P, n_m] f32 for ACT epilogues
    b32 = {}
    for n in BIAS32_LAYERS:
        nm = (LAYERS[n][1] + P - 1) // P
        b32[n] = nc.declare_dram_parameter(f"b_{n}", [P, nm], F32,
                                           isOutput=False)
    b2122 = nc.declare_dram_parameter("b2122", [1, 4096], F16, isOutput=False)
    bh1 = nc.declare_dram_parameter("bh1", [64, 1], F32, isOutput=False)
    bh2 = nc.declare_dram_parameter("bh2", [5, 1], F32, isOutput=False)
    lnp = {}
    for ln, D in LNS.items():
        nm = D // P
        for pfx in ("g", "ng", "be"):
            lnp[f"{pfx}_{ln}"] = nc.declare_dram_parameter(
                f"{pfx}_{ln}", [P, nm], F32, isOutput=False)
    outT = nc.declare_dram_parameter("outT", [4368, NB], F32, isOutput=True)

    from contextlib import ExitStack
    with tile.TileContext(nc) as tc, ExitStack() as es:
        cpool = es.enter_context(tc.tile_pool(name="const", bufs=1))
        wpool = es.enter_context(tc.tile_pool(name="wts", bufs=3))
        psum = es.enter_context(tc.tile_pool(name="psmm", bufs=4,
                                             space="PSUM"))
        pbc = es.enter_context(tc.tile_pool(name="psbc", bufs=2,
                                            space="PSUM"))
        pstat = es.enter_context(tc.tile_pool(name="psst", bufs=1,
                                              space="PSUM"))
        pmain = es.enter_context(tc.tile_pool(name="main", bufs=1))
        lv16 = pmain.tile([P, 16, NB], F16, name="lv16")
        x016 = pmain.tile([P, 16, NB], F16, name="x016")
        r2in = pmain.tile([P, 16, NB], F16, name="r2in")
        x2 = pmain.tile([P, 8, NB], F16, name="x2")

        # ---- constants ----
        ident = cpool.tile([P, P], F32, name="ident")
        make_identity(nc, ident[:])
        ones16 = cpool.tile([P, 1], F16, name="ones16")
        nc.vector.memset(ones16[:], 1.0)
        ones_row = cpool.tile([1, NB], F16, name="ones_row")
        nc.vector.memset(ones_row[:], 1.0)
        tb32 = {}
        for n in BIAS32_LAYERS:
            nm = (LAYERS[n][1] + P - 1) // P
            tb32[n] = cpool.tile([P, nm], F32, name=f"tb_{n}")
            nc.sync.dma_start(tb32[n][:], b32[n][:])
        tbh1 = cpool.tile([64, 1], F32, name="tbh1")
        nc.sync.dma_start(tbh1[:], bh1[:])
        tbh2 = cpool.tile([5, 1], F32, name="tbh2")
        nc.sync.dma_start(tbh2[:], bh2[:])
        tln = {}
        for k in lnp:
            shp = list(lnp[k].shape)
            tln[k] = cpool.tile(shp, F32, name=f"t{k}")
            nc.sync.dma_start(tln[k][:], lnp[k][:])
        twh1 = cpool.tile([P, 64], F16, name="twh1")
        nc.sync.dma_start(twh1[:], wh1[:])
        twh2 = cpool.tile([P, 5], F16, name="twh2")
        nc.sync.dma_start(twh2[:], wh2[:])

        # =================== encode ===================
        cmA = tc.tile_pool(name="pA", bufs=1)
        pA = cmA.__enter__()
        cmAt = tc.tile_pool(name="pAt", bufs=2)
        pAt = cmAt.__enter__()
        h0T = pA.tile([P, 5, NB], F16, name="h0T")
        nc.vector.memset(h0T[:, 4, :], 0.0)
        epse = cpool.tile([P, 1], F32, name="epse")
        nc.vector.memset(epse[:], 1e-12)
        for i in range(4):
            sv = pAt.tile([P, 264], F32, name=f"sv{i}", tag="sv")
            nc.sync.dma_start(sv[:], state[i * P:(i + 1) * P, :])
            h0 = pAt.tile([P, 528], F32, name=f"h0b{i}", tag="h0b")
            # stateVec
            nc.vector.tensor_copy(h0[:, 0:264], sv[:])
            # thickness even slots: x coords of pts 0..65
            t_ev_o = h0[:, 264:396].rearrange("p (i two) -> p i two", two=2)
            sv_x = sv[:, 0:132].rearrange("p (i two) -> p i two", two=2)
            nc.vector.tensor_copy(t_ev_o[:, :, 0:1], sv_x[:, :, 0:1])
            # thickness odd slots: |y_i - y_{i+66}|
            sv_y0 = sv[:, 0:132].rearrange("p (i two) -> p i two", two=2)
            sv_y1 = sv[:, 132:264].rearrange("p (i two) -> p i two", two=2)
            nc.vector.tensor_sub(t_ev_o[:, :, 1:2], sv_y0[:, :, 1:2],
                                 sv_y1[:, :, 1:2])
            nc.scalar.activation(t_ev_o[:, :, 1:2], t_ev_o[:, :, 1:2], AF.Abs)
            # vectors
            v = pAt.tile([P, 264], F32, name=f"v{i}", tag="v")
            nc.vector.tensor_sub(v[:, 0:262], sv[:, 2:264], sv[:, 0:262])
            nc.vector.tensor_sub(v[:, 262:264], sv[:, 0:2], sv[:, 262:264])
            # v2: gather of even vector indices
            v2 = pAt.tile([P, 264], F32, name=f"v2{i}", tag="v2")
            v_g = v[:, 4:264].rearrange("p (i four) -> p i four", four=4)
            for off in (0, 132):
                d = v2[:, off:off + 130].rearrange(
                    "p (i two) -> p i two", two=2)
                nc.vector.tensor_copy(d[:], v_g[:, :, 0:2])
                nc.vector.tensor_copy(v2[:, off + 130:off + 132], v[:, 0:2])
            # dot and norms
            pr = pAt.tile([P, 264], F32, name=f"pr{i}", tag="pr")
            nc.vector.tensor_mul(pr[:], v[:], v2[:])
            pr2 = pr.rearrange("p (i two) -> p i two", two=2)
            dot = pAt.tile([P, 132], F32, name=f"dot{i}", tag="dot")
            nc.vector.tensor_add(dot[:, :, None], pr2[:, :, 0:1], pr2[:, :, 1:2])
            nc.vector.tensor_mul(pr[:], v[:], v[:])
            n1 = pAt.tile([P, 132], F32, name=f"n1{i}", tag="n1")
            nc.vector.tensor_add(n1[:, :, None], pr2[:, :, 0:1], pr2[:, :, 1:2])
            n2 = pAt.tile([P, 132], F32, name=f"n2{i}", tag="n2")
            n1_g = n1[:, 2:132].rearrange("p (i two) -> p i two", two=2)
            for off in (0, 66):
                nc.vector.tensor_copy(n2[:, off:off + 65],
                                      n1_g[:, :, 0:1].rearrange(
                                          "p i one -> p (i one)"))
                nc.vector.tensor_copy(n2[:, off + 65:off + 66], n1[:, 0:1])
            den = pAt.tile([P, 132], F32, name=f"den{i}", tag="den")
            nc.vector.tensor_mul(den[:], n1[:], n2[:])
            nc.scalar.activation(den[:], den[:], AF.Sqrt, bias=epse[:])
            nc.vector.reciprocal(den[:], den[:])
            cosv = pAt.tile([P, 132], F32, name=f"cos{i}", tag="cos")
            nc.vector.tensor_mul(cosv[:], dot[:], den[:])
            nc.vector.tensor_scalar(out=cosv[:], in0=cosv[:], scalar1=1.0,
                                    scalar2=-1.0, op0=ALU.min, op1=ALU.max)
            # angles = pi/2 - atan(c / sqrt(1 - c^2))
            s1c = pAt.tile([P, 132], F32, name=f"s1c{i}", tag="s1c")
            nc.vector.tensor_mul(s1c[:], cosv[:], cosv[:])
            nc.vector.tensor_scalar(out=s1c[:], in0=s1c[:], scalar1=-1.0,
                                    scalar2=1.0, op0=ALU.mult, op1=ALU.add)
            nc.scalar.activation(s1c[:], s1c[:], AF.Sqrt, bias=epse[:])
            nc.vector.reciprocal(s1c[:], s1c[:])
            nc.vector.tensor_mul(s1c[:], cosv[:], s1c[:])
            nc.scalar.activation(s1c[:], s1c[:], AF.Arctan)
            nc.vector.tensor_scalar(out=h0[:, 396:528], in0=s1c[:],
                                    scalar1=-1.0, scalar2=HALF_PI,
                                    op0=ALU.mult, op1=ALU.add)
            # transpose h0 -> h0T
            for f in range(5):
                fw = 128 if f < 4 else 16
                ptp = pbc.tile([P, P], F32, name=f"ptp{i}_{f}", tag="bc")
                nc.tensor.transpose(ptp[:fw, :], h0[:, f * P:f * P + fw],
                                    ident[:])
                nc.scalar.activation(h0T[0:fw, f, i * P:(i + 1) * P],
                                     ptp[:fw, :], AF.Copy)

        # =================== fc1 -> h1 ===================
        h1 = pA.tile([P, 8, NB], F16, name="h1")

        def ep_fc1(t, mw, pt):
            nc.scalar.activation(h1[:, t, :], pt[:], AF.Gelu,
                                 bias=tb32["fc1"][:, t:t + 1])
        _mm_linear(nc, wpool, psum, W["fc1"], h0T, 640, 1024, ep_fc1, "fc1")

        # =================== fc21 (mu) / fc22 (logvar) + z ===================
        cmB = tc.tile_pool(name="pB", bufs=1)
        pB = cmB.__enter__()
        cmBt = tc.tile_pool(name="pBt", bufs=2)
        pBt = cmBt.__enter__()
        mu16 = pB.tile([P, 16, NB], F16, name="mu16")
        z16 = pB.tile([P, 16, NB], F16, name="z16")

        def ep_fc21(t, mw, pt):
            mo = pBt.tile([P, NB], F32, name=f"mo_{t}", tag="ocp")
            nc.scalar.activation(mo[:], pt[:], AF.Copy)
            nc.sync.dma_start(outT[ROW_MU + t * P:ROW_MU + (t + 1) * P, :],
                              mo[:])
            nc.vector.tensor_copy(mu16[:, t, :], mo[:])
        _mm_linear(nc, wpool, psum, W["fc21"], h1, 1024, 2048, ep_fc21,
                   "fc21", bias16=b2122[:, 0:2048], ones_row=ones_row)

        def ep_fc22(t, mw, pt):
            lo = pBt.tile([P, NB], F32, name=f"lo_{t}", tag="ocp")
            nc.scalar.activation(lo[:], pt[:], AF.Copy)
            nc.sync.dma_start(outT[ROW_LV + t * P:ROW_LV + (t + 1) * P, :],
                              lo[:])
            nc.vector.tensor_copy(lv16[:, t, :], lo[:])
            e05 = pBt.tile([P, NB], F32, name=f"e05_{t}", tag="e05")
            nc.scalar.activation(e05[:], pt[:], AF.Exp, scale=0.5)
            epst = pBt.tile([P, NB], F32, name=f"epst_{t}", tag="epst")
            nc.sync.dma_start(epst[:], epsT[t * P:(t + 1) * P, :])
            nc.vector.tensor_mul(e05[:], e05[:], epst[:])
            nc.vector.tensor_add(z16[:, t, :], e05[:], mu16[:, t, :])
        _mm_linear(nc, wpool, psum, W["fc22"], h1, 1024, 2048, ep_fc22,
                   "fc22", bias16=b2122[:, 2048:4096], ones_row=ones_row)

        # =================== decode: fc3 -> fc4 -> recon ===================
        g3 = pB.tile([P, 8, NB], F16, name="g3")

        def ep_fc3(t, mw, pt):
            nc.scalar.activation(g3[:, t, :], pt[:], AF.Gelu,
                                 bias=tb32["fc3"][:, t:t + 1])
        _mm_linear(nc, wpool, psum, W["fc3"], z16, 2048, 1024, ep_fc3, "fc3")

        def ep_fc4(t, mw, pt):
            rc = pBt.tile([P, NB], F32, name=f"rc{t}", tag="rc")
            nc.scalar.activation(rc[:mw, :], pt[:mw], AF.Sigmoid,
                                 bias=tb32["fc4"][:mw, t:t + 1])
            nc.sync.dma_start(
                outT[ROW_RECON + t * P:ROW_RECON + t * P + mw, :], rc[:mw, :])
        _mm_linear(nc, wpool, psum, W["fc4"], g3, 1024, 264, ep_fc4, "fc4")

        # release encode/decode pools (LIFO: pBt, pB, pAt, pA)
        cmBt.__exit__(None, None, None)
        cmB.__exit__(None, None, None)
        cmAt.__exit__(None, None, None)
        cmA.__exit__(None, None, None)

        # =================== resblock 1 ===================
        with tc.tile_pool(name="r1", bufs=1) as pr1, \
             tc.tile_pool(name="r1t", bufs=3) as pr1t:
            y1 = pr1.tile([P, 64, NB], F16, name="y1")
            s1 = pstat.tile([1, NB], F32, name="s1_r1l1", tag="s1")
            s2 = pstat.tile([1, NB], F32, name="s2_r1l1", tag="s2")

            def ep_r1f1(t, mw, pt):
                nc.scalar.activation(y1[:, t, :], pt[:], AF.Identity,
                                     bias=tb32["r1f1"][:, t:t + 1])
                y2 = pr1t.tile([P, NB], F16, name=f"y2a_{t}", tag="y2")
                nc.vector.tensor_mul(y2[:], y1[:, t, :], y1[:, t, :])
                nc.tensor.matmul(s1[:], lhsT=ones16[:], rhs=y1[:, t, :],
                                 start=(t == 0), stop=(t == 63))
                nc.tensor.matmul(s2[:], lhsT=ones16[:], rhs=y2[:],
                                 start=(t == 0), stop=(t == 63))
            _mm_linear(nc, wpool, psum, W["r1f1"], lv16, 2048, 8192,
                       ep_r1f1, "r1f1")
            _ln_stats_apply(nc, pr1t, pstat, pbc, None, y1, 64, 8192,
                            tln["g_r1l1"], tln["ng_r1l1"], tln["be_r1l1"],
                            ones16, s1, s2, "r1l1", lrelu_out=True)

            # r1f2 + ln2 + shortcut + lrelu + gelu -> x0, r2in
            yb = pr1.tile([P, 16, NB], F16, name="yb")
            s1b = pstat.tile([1, NB], F32, name="s1_r1l2", tag="s1")
            s2b = pstat.tile([1, NB], F32, name="s2_r1l2", tag="s2")

            def ep_r1f2(t, mw, pt):
                nc.scalar.activation(yb[:, t, :], pt[:], AF.Identity,
                                     bias=tb32["r1f2"][:, t:t + 1])
                y2 = pr1t.tile([P, NB], F16, name=f"y2b_{t}", tag="y2")
                nc.vector.tensor_mul(y2[:], yb[:, t, :], yb[:, t, :])
                nc.tensor.matmul(s1b[:], lhsT=ones16[:], rhs=yb[:, t, :],
                                 start=(t == 0), stop=(t == 15))
                nc.tensor.matmul(s2b[:], lhsT=ones16[:], rhs=y2[:],
                                 start=(t == 0), stop=(t == 15))
            _mm_linear(nc, wpool, psum, W["r1f2"], y1, 8192, 2048,
                       ep_r1f2, "r1f2")
            S_b, T_b = _ln_stats_apply(
                nc, pr1t, pstat, pbc, None, yb, 16, 2048,
                tln["g_r1l2"], tln["ng_r1l2"], tln["be_r1l2"],
                ones16, s1b, s2b, "r1l2", lrelu_out=False)

            for t in range(16):
                yt = yb[:, t, :]
                u = pr1t.tile([P, NB], F32, name=f"u_{t}", tag="u")
                nc.vector.scalar_tensor_tensor(
                    u[:], in0=yt, scalar=tln["g_r1l2"][:, t:t + 1],
                    in1=S_b[:], op0=ALU.mult, op1=ALU.mult)
                nc.vector.scalar_tensor_tensor(
                    u[:], in0=T_b[:], scalar=tln["ng_r1l2"][:, t:t + 1],
                    in1=u[:], op0=ALU.mult, op1=ALU.add)
                # shortcut: + logvar (fp16 copy), bias beta via ACT, lrelu
                nc.vector.tensor_add(u[:], u[:], lv16[:, t, :])
                nc.scalar.activation(u[:], u[:], AF.Lrelu,
                                     bias=tln["be_r1l2"][:, t:t + 1],
                                     alpha=0.01)
                x0f = pr1t.tile([P, NB], F32, name=f"x0f_{t}", tag="x0f")
                nc.scalar.activation(x0f[:], u[:], AF.Gelu)
                nc.vector.tensor_copy(x016[:, t, :], x0f[:])
                nc.vector.tensor_add(r2in[:, t, :], x0f[:], lv16[:, t, :])

        # =================== resblock 2 ===================
        with tc.tile_pool(name="r2", bufs=1) as pr2, \
             tc.tile_pool(name="r2t", bufs=3) as pr2t:
            ya = pr2.tile([P, 32, NB], F16, name="ya")
            s1c_ = pstat.tile([1, NB], F32, name="s1_r2l1", tag="s1")
            s2c_ = pstat.tile([1, NB], F32, name="s2_r2l1", tag="s2")

            def ep_r2f1(t, mw, pt):
                nc.scalar.activation(ya[:, t, :], pt[:], AF.Identity,
                                     bias=tb32["r2f1"][:, t:t + 1])
                y2 = pr2t.tile([P, NB], F16, name=f"y2c_{t}", tag="y2")
                nc.vector.tensor_mul(y2[:], ya[:, t, :], ya[:, t, :])
                nc.tensor.matmul(s1c_[:], lhsT=ones16[:], rhs=ya[:, t, :],
                                 start=(t == 0), stop=(t == 31))
                nc.tensor.matmul(s2c_[:], lhsT=ones16[:], rhs=y2[:],
                                 start=(t == 0), stop=(t == 31))
            _mm_linear(nc, wpool, psum, W["r2f1"], r2in, 2048, 4096,
                       ep_r2f1, "r2f1")
            _ln_stats_apply(nc, pr2t, pstat, pbc, None, ya, 32, 4096,
                            tln["g_r2l1"], tln["ng_r2l1"], tln["be_r2l1"],
                            ones16, s1c_, s2c_, "r2l1", lrelu_out=True)

            yd = pr2.tile([P, 16, NB], F16, name="yd")
            s1d = pstat.tile([1, NB], F32, name="s1_r2l2", tag="s1")
            s2d = pstat.tile([1, NB], F32, name="s2_r2l2", tag="s2")

            def ep_r2f2(t, mw, pt):
                nc.scalar.activation(yd[:, t, :], pt[:], AF.Identity,
                                     bias=tb32["r2f2"][:, t:t + 1])
                y2 = pr2t.tile([P, NB], F16, name=f"y2d_{t}", tag="y2")
                nc.vector.tensor_mul(y2[:], yd[:, t, :], yd[:, t, :])
                nc.tensor.matmul(s1d[:], lhsT=ones16[:], rhs=yd[:, t, :],
                                 start=(t == 0), stop=(t == 15))
                nc.tensor.matmul(s2d[:], lhsT=ones16[:], rhs=y2[:],
                                 start=(t == 0), stop=(t == 15))
            _mm_linear(nc, wpool, psum, W["r2f2"], ya, 4096, 2048,
                       ep_r2f2, "r2f2")
            S_b2, T_b2 = _ln_stats_apply(
                nc, pr2t, pstat, pbc, None, yd, 16, 2048,
                tln["g_r2l2"], tln["ng_r2l2"], tln["be_r2l2"],
                ones16, s1d, s2d, "r2l2", lrelu_out=False)

            # glu input = x1 + x0 where x1 = gelu(lrelu(ln2 + r2in))
            gsum = pr2.tile([P, 16, NB], F16, name="gsum")
            for t in range(16):
                yt = yd[:, t, :]
                u = pr2t.tile([P, NB], F32, name=f"u2_{t}", tag="u")
                nc.vector.scalar_tensor_tensor(
                    u[:], in0=yt, scalar=tln["g_r2l2"][:, t:t + 1],
                    in1=S_b2[:], op0=ALU.mult, op1=ALU.mult)
                nc.vector.scalar_tensor_tensor(
                    u[:], in0=T_b2[:], scalar=tln["ng_r2l2"][:, t:t + 1],
                    in1=u[:], op0=ALU.mult, op1=ALU.add)
                nc.vector.tensor_add(u[:], u[:], r2in[:, t, :])
                nc.scalar.activation(u[:], u[:], AF.Lrelu,
                                     bias=tln["be_r2l2"][:, t:t + 1],
                                     alpha=0.01)
                nc.scalar.activation(u[:], u[:], AF.Gelu)
                nc.vector.tensor_add(gsum[:, t, :], u[:], x016[:, t, :])

            # GLU: x2 = vals * sigmoid(gates)
            for t in range(8):
                sg = pr2t.tile([P, NB], F32, name=f"sg_{t}", tag="sg")
                nc.scalar.activation(sg[:], gsum[:, 8 + t, :], AF.Sigmoid)
                nc.vector.tensor_mul(x2[:, t, :], gsum[:, t, :], sg[:])

        # =================== fc -> x3, heads ===================
        ph = es.enter_context(tc.tile_pool(name="ph", bufs=1))
        x3 = ph.tile([P, 1, NB], F16, name="x3")

        def ep_fc(t, mw, pt):
            nc.scalar.activation(x3[:, 0, :], pt[:], AF.Gelu,
                                 bias=tb32["fc"][:, 0:1])
        _mm_linear(nc, wpool, psum, W["fc"], x2, 1024, 128, ep_fc, "fc")

        # heads level 1: [128 -> 64] (gelu rows 0..31, relu rows 32..63)
        hs16 = ph.tile([P, NB], F16, name="hs16")
        nc.vector.memset(hs16[:], 0.0)
        ph1 = pbc.tile([P, NB], F32, name="ph1", tag="bc")
        nc.tensor.matmul(ph1[:64, :], lhsT=twh1[:], rhs=x3[:, 0, :],
                         start=True, stop=True)
        nc.scalar.activation(hs16[0:32, :], ph1[0:32, :], AF.Gelu,
                             bias=tbh1[0:32, :])
        nc.scalar.activation(hs16[32:64, :], ph1[32:64, :], AF.Relu,
                             bias=tbh1[32:64, :])

        # softmax over rows 0..2
        e3 = ph.tile([3, NB], F32, name="e3")
        nc.scalar.activation(e3[:], hs16[0:3, :], AF.Exp)
        ones3 = cpool.tile([3, 1], F32, name="ones3")
        nc.vector.memset(ones3[:], 1.0)
        pS3 = pbc.tile([P, NB], F32, name="pS3", tag="bc")
        nc.tensor.matmul(pS3[0:1, :], lhsT=ones3[:], rhs=e3[:],
                         start=True, stop=True)
        rS = ph.tile([1, NB], F32, name="rS")
        nc.vector.reciprocal(rS[:], pS3[0:1, :])
        ones13 = cpool.tile([1, 3], F32, name="ones13")
        nc.vector.memset(ones13[:], 1.0)
        pB3 = pbc.tile([P, NB], F32, name="pB3", tag="bc")
        nc.tensor.matmul(pB3[0:3, :], lhsT=ones13[:], rhs=rS[:],
                         start=True, stop=True)
        act3 = ph.tile([3, NB], F32, name="act3")
        nc.vector.tensor_mul(act3[:], e3[:], pB3[0:3, :])
        nc.sync.dma_start(outT[ROW_ACTION:ROW_ACTION + 3, :], act3[:])

        # heads level 2: [128(padded 48) -> 5]
        ph2 = pbc.tile([P, NB], F32, name="ph2", tag="bc")
        nc.tensor.matmul(ph2[0:5, :], lhsT=twh2[:], rhs=hs16[:],
                         start=True, stop=True)
        # row 0: p2_0 = psig(t) = s*(1.1 - 0.1 s);  rows 1..4: C*ptanh
        # (compute both paths on all 5 rows: partition starts must be 0)
        sgm = ph.tile([5, NB], F32, name="sgm")
        nc.scalar.activation(sgm[:], ph2[0:5, :], AF.Sigmoid,
                             bias=tbh2[0:5, :])
        q0 = ph.tile([5, NB], F32, name="q0")
        nc.vector.tensor_scalar(out=q0[:], in0=sgm[:], scalar1=-0.1,
                                scalar2=1.1, op0=ALU.mult, op1=ALU.add)
        nc.vector.tensor_mul(q0[:], q0[:], sgm[:])
        nc.sync.dma_start(outT[ROW_P20:ROW_P20 + 1, :], q0[0:1, :])
        th = ph.tile([5, NB], F32, name="th")
        nc.scalar.activation(th[:], ph2[0:5, :], AF.Tanh, bias=tbh2[0:5, :])
        q1 = ph.tile([5, NB], F32, name="q1")
        nc.vector.tensor_scalar(out=q1[:], in0=th[:], scalar1=-0.0002,
                                scalar2=0.002, op0=ALU.mult, op1=ALU.add)
        nc.vector.tensor_mul(q1[:], q1[:], th[:])
        nc.sync.dma_start(outT[ROW_P30:ROW_P30 + 4, :], q1[1:5, :])

    return nc


# ---------------------------------------------------------------------------
# Host side
# ---------------------------------------------------------------------------
def _pack_bias(b, n_m):
    v = np.zeros((P, n_m), np.float32)
    flat = np.asarray(b, np.float32)
    for t in range(n_m):
        w = min(P, flat.shape[0] - t * P)
        v[:w, t] = flat[t * P:t * P + w]
    return v


def _prep_params(p):
    f16 = np.float16
    d = {}

    def wt(src, kpad=None):
        w = np.asarray(p[src + "_w"], np.float32).T  # [in, out]
        if kpad is not None and w.shape[0] < kpad:
            w = np.vstack([w, np.zeros((kpad - w.shape[0], w.shape[1]),
                                       np.float32)])
        K, M = w.shape
        nm = (M + P - 1) // P
        if M < nm * P:
            w = np.hstack([w, np.zeros((K, nm * P - M), np.float32)])
        nk = K // P
        # [nm, P(k within tile), nk*P(m)] with per-partition contiguity
        wt_ = w.reshape(nk, P, nm, P).transpose(2, 1, 0, 3).reshape(
            nm, P, nk * P)
        return np.ascontiguousarray(wt_).astype(f16)

    d["w_fc1"] = wt("fc1", 640)
    d["w_fc21"] = wt("fc21")
    d["w_fc22"] = wt("fc22")
    d["w_fc3"] = wt("fc3")
    d["w_fc4"] = wt("fc4")
    d["w_r1f1"] = wt("r1_fc1")
    d["w_r1f2"] = wt("r1_fc2")
    d["w_r2f1"] = wt("r2_fc1")
    d["w_r2f2"] = wt("r2_fc2")
    d["w_fc"] = wt("fc")
    for n, src in [("fc1", "fc1"), ("fc3", "fc3"), ("fc4", "fc4"),
                   ("r1f1", "r1_fc1"), ("r1f2", "r1_fc2"),
                   ("r2f1", "r2_fc1"), ("r2f2", "r2_fc2"), ("fc", "fc")]:
        n_m = (LAYERS[n][1] + P - 1) // P
        d[f"b_{n}"] = _pack_bias(p[src + "_b"], n_m)
    d["b2122"] = np.concatenate(
        [np.asarray(p["fc21_b"], np.float32),
         np.asarray(p["fc22_b"], np.float32)])[None, :].astype(f16)
    # LN params
    for ln, src in [("r1l1", "r1_ln1"), ("r1l2", "r1_ln2"),
                    ("r2l1", "r2_ln1"), ("r2l2", "r2_ln2")]:
        D = LNS[ln]
        g = np.asarray(p[src + "_g"], np.float32)
        be = np.asarray(p[src + "_b"], np.float32)
        d[f"g_{ln}"] = _pack_bias(g, D // P)
        d[f"ng_{ln}"] = -d[f"g_{ln}"]
        d[f"be_{ln}"] = _pack_bias(be, D // P)
    # heads: fused level-1 weight [128, 51] = [act(3) | p1_0 | p1_1 | p1_2]
    wh1 = np.zeros((P, 64), np.float32)
    wh1[:, 0:3] = np.asarray(p["act_w"], np.float32).T
    wh1[:, 3:19] = np.asarray(p["p1_0_w"], np.float32).T
    wh1[:, 32:48] = np.asarray(p["p1_1_w"], np.float32).T
    wh1[:, 48:64] = np.asarray(p["p1_2_w"], np.float32).T
    d["w_h1"] = wh1.astype(f16)
    bh1 = np.zeros((64,), np.float32)
    bh1[0:3] = np.asarray(p["act_b"], np.float32)
    bh1[3:19] = np.asarray(p["p1_0_b"], np.float32)
    bh1[32:48] = np.asarray(p["p1_1_b"], np.float32)
    bh1[48:64] = np.asarray(p["p1_2_b"], np.float32)
    d["bh1"] = bh1[:, None].astype(np.float32)
    # level-2 block diagonal [128, 5]; rows 3..18 p1_0, 19..34 p1_1, 35..50 p1_2
    wh2 = np.zeros((P, 5), np.float32)
    wh2[3:19, 0] = np.asarray(p["p2_0_w"], np.float32)[0]
    wh2[3:19, 1] = np.asarray(p["p3_0_w"], np.float32)[0]
    wh2[3:19, 2] = np.asarray(p["p4_0_w"], np.float32)[0]
    wh2[32:48, 3] = np.asarray(p["p2_1_w"], np.float32)[0]
    wh2[48:64, 4] = np.asarray(p["p2_2_w"], np.float32)[0]
    d["w_h2"] = wh2.astype(f16)
    bh2 = np.array([p["p2_0_b"][0], p["p3_0_b"][0], p["p4_0_b"][0],
                    p["p2_1_b"][0], p["p2_2_b"][0]], np.float32)
    d["bh2"] = bh2[:, None]
    return d


_CACHED = {}


def kernel(state, eps, params):
    state = np.asarray(state, np.float32)
    eps = np.asarray(eps, np.float32)
    B = state.shape[0]
    per = B // NCORES
    assert per == NB, (B, NB)

    shared = _prep_params(params)
    in_maps = []
    for i in range(NCORES):
        m = dict(shared)
        m["state"] = np.ascontiguousarray(
            state[i * per:(i + 1) * per].reshape(per, 264))
        m["epsT"] = np.ascontiguousarray(eps[i * per:(i + 1) * per].T)
        in_maps.append(m)

    if "nc" not in _CACHED:
        _CACHED["nc"] = build_graph()
    nc = _CACHED["nc"]
    res = run_bass_kernel_spmd(nc, in_maps, core_ids=list(range(NCORES)))
    out = np.empty((B, 4368), np.float32)
    for i in range(NCORES):
        out[i * per:(i + 1) * per] = res.results[i]["outT"].T
    return out
